# revision 3
# baseline (speedup 1.0000x reference)
"""Differentiable VACF on 8 Trainium2 NeuronCores.

vel [T=10000, N=1000, 3] f32 -> vacf [100] f32 where
vacf[t] = mean(flat[t:] * flat[:-t]) with flat = vel.reshape(T, 3000).

Strategy (sharded over the atom dimension, 125 atoms = 375 channels per core):
  vacf[t]*count = sum_i sum_c flat[i, c] * flat[i+t, c]  -- a channel-summed
  time autocorrelation. Per core, channels are split into 3 groups of 125
  (padded to 128 partitions) laid out channel-major [128, T]. For each
  128-wide time block the PE computes the Gram block
      G[m, n] += sum_k X[k, i0+m] * X[k, i0+n],  n in [0, 227)
  accumulated over every block and group into a single PSUM [128, 227]
  accumulation group (237 matmuls). The lag sums are the diagonals
  S[t] = sum_m G[m, m+t], extracted on host from the tiny [128, 227] output,
  summed across cores, and normalized.
"""

import numpy as np
import ml_dtypes

import concourse.bass as bass
import concourse.tile as tile
import concourse.mybir as mybir
from concourse import bacc
from concourse.bass_utils import run_bass_kernel_spmd

N_CORES = 8
T = 10000
N_ATOMS = 1000
LAGS = 100
ATOMS_PER_CORE = N_ATOMS // N_CORES       # 125
CH_PER_GROUP = ATOMS_PER_CORE             # 125 channels per group (x3 groups)
GROUPS = 3
BLK = 128                                  # time-block rows (lhsT cols)
RHS_N = BLK + LAGS - 1                     # 227 rhs cols per matmul
SEG_BLOCKS = 20
SEG = SEG_BLOCKS * BLK                     # 2560 time steps per DMA segment
SEG_COLS = SEG + 256                       # 2816 (covers the rhs overhang)
N_SEGS = 4
N_BLOCKS = 79                              # ceil(10000 / 128)
T_PAD = N_SEGS * SEG + 256                 # 10496

DT = mybir.dt.bfloat16
NP_DT = ml_dtypes.bfloat16

_cache = {}


def _build():
    if "nc" in _cache:
        return _cache["nc"]
    nc = bacc.Bacc("TRN2", debug=False, num_devices=N_CORES)
    # contiguous per-(segment, group) blocks so each load DMA is one dense
    # 704KB stream (strided DRAM reads measured only ~207 GB/s)
    x = nc.dram_tensor(
        "x", [N_SEGS, GROUPS, 128, SEG_COLS], DT, kind="ExternalInput"
    )
    g_out = nc.dram_tensor(
        "g_out", [128, RHS_N], mybir.dt.float32, kind="ExternalOutput"
    )

    with tile.TileContext(nc) as tc:
        with (
            tc.tile_pool(name="seg", bufs=N_SEGS * GROUPS) as seg_pool,
            tc.tile_pool(name="psum", bufs=1, space="PSUM") as psum_pool,
            tc.tile_pool(name="out", bufs=1) as out_pool,
        ):
            tiles = {}
            for s in range(N_SEGS):
                for g in range(GROUPS):
                    t = seg_pool.tile([128, SEG_COLS], DT)
                    # alternate the two HWDGE rings (SP / ACT) for overlap
                    dma_eng = nc.sync if (s * GROUPS + g) % 2 == 0 else nc.scalar
                    dma_eng.dma_start(out=t[:], in_=x[s, g])
                    tiles[(s, g)] = t

            psum = psum_pool.tile([128, RHS_N], mybir.dt.float32)
            n_mm = N_BLOCKS * GROUPS
            idx = 0
            for b in range(N_BLOCKS):
                s = b // SEG_BLOCKS
                lo = b * BLK - s * SEG
                for g in range(GROUPS):
                    nc.tensor.matmul(
                        psum[:, :],
                        lhsT=tiles[(s, g)][:, lo : lo + BLK],
                        rhs=tiles[(s, g)][:, lo : lo + RHS_N],
                        start=(idx == 0),
                        stop=(idx == n_mm - 1),
                    )
                    idx += 1

            out_sb = out_pool.tile([128, RHS_N], mybir.dt.float32)
            nc.vector.tensor_copy(out_sb[:], psum[:])
            nc.sync.dma_start(out=g_out[:], in_=out_sb[:])

    nc.compile()
    _cache["nc"] = nc
    return nc


def _shard_inputs(vel):
    in_maps = []
    for c in range(N_CORES):
        a0 = c * ATOMS_PER_CORE
        A = np.ascontiguousarray(
            vel[:, a0 : a0 + ATOMS_PER_CORE, :]
        ).reshape(T, ATOMS_PER_CORE * 3)
        Xt = np.zeros((GROUPS, 128, T_PAD), dtype=NP_DT)
        for g in range(GROUPS):
            Xt[g, :CH_PER_GROUP, :T] = (
                A[:, g * CH_PER_GROUP : (g + 1) * CH_PER_GROUP].T.astype(NP_DT)
            )
        X = np.empty((N_SEGS, GROUPS, 128, SEG_COLS), dtype=NP_DT)
        for s in range(N_SEGS):
            X[s] = Xt[:, :, s * SEG : s * SEG + SEG_COLS]
        in_maps.append({"x": X})
    return in_maps


def run(vel, vacf_window, trace=False):
    vel = np.asarray(vel, dtype=np.float32)
    W = int(vacf_window)
    assert vel.shape == (T, N_ATOMS, 3), vel.shape
    assert 1 <= W <= LAGS, W

    nc = _build()
    in_maps = _shard_inputs(vel)
    res = run_bass_kernel_spmd(
        nc, in_maps, list(range(N_CORES)), trace=trace
    )

    S = np.zeros(W, dtype=np.float64)
    for c in range(N_CORES):
        G = res.results[c]["g_out"].astype(np.float64)
        for t in range(W):
            S[t] += np.trace(G, offset=t)
    counts = (T - np.arange(W)).astype(np.float64) * (N_ATOMS * 3)
    out = (S / counts).astype(np.float32)
    return out, res


def kernel(vel, vacf_window):
    out, _ = run(vel, vacf_window, trace=False)
    return out


# revision 5
# speedup vs baseline: 1.0964x; 1.0964x over previous
"""Differentiable VACF on 8 Trainium2 NeuronCores.

vel [T=10000, N=1000, 3] f32 -> vacf [100] f32 where
vacf[t] = mean(flat[t:] * flat[:-t]) with flat = vel.reshape(T, 3000).

Strategy (sharded over the atom dimension, 125 atoms = 375 channels per core):
  vacf[t]*count = sum_i sum_c flat[i, c] * flat[i+t, c]  -- a channel-summed
  time autocorrelation. Per core, channels are split into 3 groups of 125
  (padded to 128 partitions) laid out channel-major [128, T]. For each
  128-wide time block the PE computes the Gram block
      G[m, n] += sum_k X[k, i0+m] * X[k, i0+n],  n in [0, 227)
  accumulated over every block and group into a single PSUM [128, 227]
  accumulation group (237 matmuls). The lag sums are the diagonals
  S[t] = sum_m G[m, m+t], extracted on host from the tiny [128, 227] output,
  summed across cores, and normalized.
"""

import numpy as np
import ml_dtypes

import concourse.bass as bass
import concourse.tile as tile
import concourse.mybir as mybir
from concourse import bacc
from concourse.bass_utils import run_bass_kernel_spmd

N_CORES = 8
T = 10000
N_ATOMS = 1000
LAGS = 100
ATOMS_PER_CORE = N_ATOMS // N_CORES       # 125
CH_PER_GROUP = ATOMS_PER_CORE             # 125 channels per group (x3 groups)
GROUPS = 3
BLK = 128                                  # time-block rows (lhsT cols)
RHS_N = BLK + LAGS - 1                     # 227 rhs cols per matmul
SEG_BLOCKS = [4, 8, 12, 16, 20, 19]        # staircase: small first seg so the
N_SEGS = len(SEG_BLOCKS)                   # PE can start early
SEG_START = [0]
for _nb in SEG_BLOCKS:
    SEG_START.append(SEG_START[-1] + _nb)
N_BLOCKS = SEG_START[-1]                   # 79 = ceil(10000 / 128)
SEG_W = [(nb - 1) * BLK + RHS_N for nb in SEG_BLOCKS]  # exact tile widths
T_PAD = 10240                              # last seg: 7680 + 2531 <= 10240
N_WARMUP = 26                              # dummy MMs to lift HAM to 2.4 GHz

DT = mybir.dt.bfloat16
NP_DT = ml_dtypes.bfloat16

_cache = {}


def _build():
    if "nc" in _cache:
        return _cache["nc"]
    nc = bacc.Bacc("TRN2", debug=False, num_devices=N_CORES)
    # one contiguous DRAM block per (segment, group) so each load DMA is a
    # dense stream (strided DRAM reads measured only ~207 GB/s)
    xs = [
        nc.dram_tensor(f"x{s}", [GROUPS, 128, SEG_W[s]], DT, kind="ExternalInput")
        for s in range(N_SEGS)
    ]
    g_out = nc.dram_tensor(
        "g_out", [128, RHS_N], mybir.dt.float32, kind="ExternalOutput"
    )

    with tile.TileContext(nc) as tc:
        with (
            tc.tile_pool(name="seg", bufs=1) as seg_pool,
            tc.tile_pool(name="warm", bufs=1) as warm_pool,
            tc.tile_pool(name="psum", bufs=1, space="PSUM") as psum_pool,
            tc.tile_pool(name="wpsum", bufs=1, space="PSUM") as wpsum_pool,
            tc.tile_pool(name="out", bufs=1) as out_pool,
        ):
            # PE warmup: harmless matmuls on a zeroed tile into a scratch
            # bank while the first loads are in flight (HAM un-throttles
            # after ~3.4us of sustained PE activity).
            warm = warm_pool.tile([128, RHS_N + BLK], DT)
            nc.gpsimd.memset(warm[:], 0.0)
            wpsum = wpsum_pool.tile([128, RHS_N], mybir.dt.float32)
            for w in range(N_WARMUP):
                nc.tensor.matmul(
                    wpsum[:, :],
                    lhsT=warm[:, :BLK],
                    rhs=warm[:, BLK : BLK + RHS_N],
                    start=True,
                    stop=True,
                )

            tiles = {}
            for s in range(N_SEGS):
                for g in range(GROUPS):
                    t = seg_pool.tile([128, SEG_W[s]], DT, tag=f"seg{s}_{g}")
                    # alternate the two HWDGE rings (SP / ACT) for overlap
                    dma_eng = nc.sync if (s * GROUPS + g) % 2 == 0 else nc.scalar
                    dma_eng.dma_start(out=t[:], in_=xs[s][g])
                    tiles[(s, g)] = t

            psum = psum_pool.tile([128, RHS_N], mybir.dt.float32)
            n_mm = N_BLOCKS * GROUPS
            idx = 0
            for s in range(N_SEGS):
                for b in range(SEG_BLOCKS[s]):
                    lo = b * BLK
                    for g in range(GROUPS):
                        nc.tensor.matmul(
                            psum[:, :],
                            lhsT=tiles[(s, g)][:, lo : lo + BLK],
                            rhs=tiles[(s, g)][:, lo : lo + RHS_N],
                            start=(idx == 0),
                            stop=(idx == n_mm - 1),
                        )
                        idx += 1

            out_sb = out_pool.tile([128, RHS_N], mybir.dt.float32)
            nc.vector.tensor_copy(out_sb[:], psum[:])
            nc.sync.dma_start(out=g_out[:], in_=out_sb[:])

    nc.compile()
    _cache["nc"] = nc
    return nc


def _shard_inputs(vel):
    in_maps = []
    for c in range(N_CORES):
        a0 = c * ATOMS_PER_CORE
        A = np.ascontiguousarray(
            vel[:, a0 : a0 + ATOMS_PER_CORE, :]
        ).reshape(T, ATOMS_PER_CORE * 3)
        Xt = np.zeros((GROUPS, 128, T_PAD), dtype=NP_DT)
        for g in range(GROUPS):
            Xt[g, :CH_PER_GROUP, :T] = (
                A[:, g * CH_PER_GROUP : (g + 1) * CH_PER_GROUP].T.astype(NP_DT)
            )
        in_map = {}
        for s in range(N_SEGS):
            c0 = SEG_START[s] * BLK
            in_map[f"x{s}"] = np.ascontiguousarray(Xt[:, :, c0 : c0 + SEG_W[s]])
        in_maps.append(in_map)
    return in_maps


def run(vel, vacf_window, trace=False):
    vel = np.asarray(vel, dtype=np.float32)
    W = int(vacf_window)
    assert vel.shape == (T, N_ATOMS, 3), vel.shape
    assert 1 <= W <= LAGS, W

    nc = _build()
    in_maps = _shard_inputs(vel)
    res = run_bass_kernel_spmd(
        nc, in_maps, list(range(N_CORES)), trace=trace
    )

    S = np.zeros(W, dtype=np.float64)
    for c in range(N_CORES):
        G = res.results[c]["g_out"].astype(np.float64)
        for t in range(W):
            S[t] += np.trace(G, offset=t)
    counts = (T - np.arange(W)).astype(np.float64) * (N_ATOMS * 3)
    out = (S / counts).astype(np.float32)
    return out, res


def kernel(vel, vacf_window):
    out, _ = run(vel, vacf_window, trace=False)
    return out
